# revision 1
# baseline (speedup 1.0000x reference)
"""Multi-head attention (B=2, S=2048, D=1024, H=16, dh=64) on 8 TRN2 NeuronCores.

Sharding: tensor-parallel over heads — 2 heads per core. Each core computes
Q/K/V projections for its 2 heads, full attention over S=2048, and a partial
output projection (its 128 rows of Wo). Host sums the 8 partial outputs + bo.

Per-core dataflow (all matmuls in float32r, 1 cyc/row at N=512):
  A) QKV^T:   psum[dh2=128, tok 512] = sum_k Wp_k[128,128].T @ xT_k[128,512]
  T) V^T -> V via PE transpose (ctx matmul needs t on partitions)
  B) scoresT: psum[t=128, s 512] = K^T_h[64,128].T @ Q^T_h[64,512]  (2 heads
     row-tiled into one [128,1024] psum tile)
  E) expT = exp(0.125 * scoresT)  (ACT, scale folded; no max-subtraction —
     scores are O(1) by construction)
  C) ctx^T aug: psum[65, 512] = sum_t [V_h|1][128,65].T @ expT[128,512]
     row 64 = softmax denominator l
  N) ctxn = ctx * (1/l)  (DVE recip + DMA partition-broadcast + DVE mult)
  D) out[s 128, d 512] = ctxn[:,s128][128,128].T @ Wo[128,512]
"""

import numpy as np

import concourse.bacc as bacc
import concourse.mybir as mybir
import concourse.tile as tile
from concourse.bass_utils import run_bass_kernel_spmd

F32 = mybir.dt.float32
F32R = mybir.dt.float32r

B, S, D, H, DH = 2, 2048, 1024, 16, 64
TOK = B * S          # 4096
DH2 = 2 * DH         # 128 (two heads per core)
NCORES = 8
SC = 512             # s-chunk
NSC = S // SC        # 4 s-chunks per batch
NT = S // 128        # 16 t-tiles per batch
NKT = D // 128       # 8 k-tiles of contraction
NCH = TOK // SC      # 8 token chunks for stage A


def build_bass():
    nc = bacc.Bacc(None, target_bir_lowering=False)

    xT = nc.dram_tensor("xT", [D, TOK], F32, kind="ExternalInput")
    wq = nc.dram_tensor("wq", [D, DH2], F32, kind="ExternalInput")
    wk = nc.dram_tensor("wk", [D, DH2], F32, kind="ExternalInput")
    wv = nc.dram_tensor("wv", [D, DH2], F32, kind="ExternalInput")
    bqkv = nc.dram_tensor("bqkv", [3, DH2], F32, kind="ExternalInput")
    wo = nc.dram_tensor("wo", [DH2, D], F32, kind="ExternalInput")
    ones = nc.dram_tensor("ones", [128, 32], F32, kind="ExternalInput")
    iden = nc.dram_tensor("iden", [128, 128], F32, kind="ExternalInput")
    out = nc.dram_tensor("out", [TOK, D], F32, kind="ExternalOutput")

    with tile.TileContext(nc) as tc:
        with (
            tc.tile_pool(name="persist", bufs=1) as persist,
            tc.tile_pool(name="xin", bufs=10) as xin,
            tc.tile_pool(name="exps", bufs=8) as exps,
            tc.tile_pool(name="work", bufs=2) as work,
            tc.tile_pool(name="ost", bufs=2) as ost,
            tc.tile_pool(name="ps_big", bufs=2, space="PSUM") as ps_big,
            tc.tile_pool(name="ps_ctx", bufs=2, space="PSUM") as ps_ctx,
            tc.tile_pool(name="ps_out", bufs=2, space="PSUM") as ps_out,
            tc.tile_pool(name="dscratch", bufs=2, space="DRAM") as dscratch,
        ):
            # ---- constants / persistent tiles ----
            w_sb = persist.tile([128, 3, NKT, DH2], F32R, tag="w")
            for i, w in enumerate((wq, wk, wv)):
                nc.sync.dma_start(
                    out=w_sb[:, i, :, :],
                    in_=w.rearrange("(t p) m -> p t m", p=128).bitcast(F32R),
                )
            b_sb = persist.tile([128, 3], F32, tag="b")
            nc.gpsimd.dma_start(out=b_sb[:], in_=bqkv.rearrange("q p -> p q"))
            wo_sb = persist.tile([128, D], F32R, tag="wo")
            nc.sync.dma_start(out=wo_sb[:], in_=wo[:, :].bitcast(F32R))
            ident = persist.tile([128, 128], F32R, tag="id")
            nc.sync.dma_start(out=ident[:], in_=iden[:, :].bitcast(F32R))

            qT = persist.tile([128, TOK], F32R, tag="qT")
            kT = persist.tile([128, TOK], F32R, tag="kT")
            vT = persist.tile([128, TOK], F32R, tag="vT")
            # V in [t, e] layout, 130 = [V_h0(64) | 1 | V_h1(64) | 1]
            v_sb = persist.tile([128, TOK // 128, 130], F32R, tag="v")
            import concourse.bass as bass_mod
            o1 = ones[0:1, 0:TOK // 128]
            ones_b = bass_mod.AP(tensor=o1.tensor, offset=o1.offset,
                                 ap=[[0, 128], [1, TOK // 128]]).bitcast(F32R)
            nc.gpsimd.dma_start(out=v_sb[:, :, 64], in_=ones_b)
            nc.gpsimd.dma_start(out=v_sb[:, :, 129], in_=ones_b)

            xTv = xT.rearrange("(t p) n -> p t n", p=128)

            # ---- stage A: QKV projections ----
            for ch in range(NCH):
                c0 = ch * SC
                xts = []
                for kt in range(NKT):
                    x_t = xin.tile([128, SC], F32R, tag="x")
                    eng = nc.sync if kt % 2 == 0 else nc.gpsimd
                    eng.dma_start(out=x_t[:], in_=xTv[:, kt, c0:c0 + SC].bitcast(F32R))
                    xts.append(x_t)
                ps_qk = ps_big.tile([128, 1024], F32, tag="big")
                ps_v = ps_big.tile([128, 1024], F32, tag="big")
                dests = (qT, kT, vT)
                outs = (ps_qk[:, 0:512], ps_qk[:, 512:1024], ps_v[:, 0:512])
                for p in range(3):
                    for kt in range(NKT):
                        nc.tensor.matmul(
                            outs[p],
                            w_sb[:, p, kt, :],
                            xts[kt][:],
                            start=(kt == 0), stop=(kt == NKT - 1),
                        )
                for p in range(3):
                    nc.vector.tensor_scalar_add(
                        dests[p][:, c0:c0 + SC], outs[p], b_sb[:, p:p + 1]
                    )

            # ---- stage T: transpose V^T -> v_sb ----
            for blk in range(TOK // 128):
                ps_t = ps_ctx.tile([128, 128], F32R, tag="ctx")
                nc.tensor.transpose(ps_t[:], vT[:, blk * 128:(blk + 1) * 128], ident[:])
                nc.vector.tensor_copy(v_sb[:, blk, 0:64], ps_t[:, 0:64])
                nc.vector.tensor_copy(v_sb[:, blk, 65:129], ps_t[:, 64:128])

            # ---- main loop over (batch, s-chunk) ----
            for b in range(B):
                for sc in range(NSC):
                    q0 = b * S + sc * SC
                    # stage B + E: scores^T and exp, per t-tile
                    etiles = []
                    for tq in range(8):  # octets of t-tiles for dep granularity
                        e_t = exps.tile([128, 2, 1024], F32R, tag="e")
                        etiles.append(e_t)
                    for tt in range(NT):
                        t0 = b * S + tt * 128
                        ps_s = ps_big.tile([128, 1024], F32, tag="big")
                        nc.tensor.matmul(
                            ps_s[:, 0:512],
                            kT[0:64, t0:t0 + 128],
                            qT[0:64, q0:q0 + SC],
                            start=True, stop=True,
                        )
                        nc.tensor.matmul(
                            ps_s[:, 512:1024],
                            kT[64:128, t0:t0 + 128],
                            qT[64:128, q0:q0 + SC],
                            start=True, stop=True,
                        )
                        nc.scalar.activation(
                            etiles[tt // 2][:, tt % 2, :], ps_s[:],
                            mybir.ActivationFunctionType.Exp, scale=0.125,
                        )
                    # stage C + N: ctx, denominators, normalize
                    ctxn = work.tile([128, SC], F32R, tag="ctxn")
                    for h in range(2):
                        ps_c = ps_ctx.tile([65, SC], F32, tag="ctx")
                        for tt in range(NT):
                            nc.tensor.matmul(
                                ps_c[:],
                                v_sb[:, b * NT + tt, h * 65:h * 65 + 65],
                                etiles[tt // 2][:, tt % 2, h * 512:h * 512 + 512],
                                start=(tt == 0), stop=(tt == NT - 1),
                            )
                        rb = work.tile([1, SC], F32, tag="rb")
                        nc.vector.reciprocal(rb[:], ps_c[64:65, :])
                        rd = dscratch.tile([1, SC], F32, tag="rd")
                        nc.sync.dma_start(out=rd[:], in_=rb[:])
                        rbb = work.tile([64, SC], F32, tag="rbb")
                        nc.gpsimd.dma_start(out=rbb[:], in_=rd[:].to_broadcast([64, SC]))
                        if h == 0:
                            nc.vector.tensor_mul(ctxn[0:64, :], ps_c[0:64, :], rbb[:])
                        else:
                            tmp = work.tile([64, SC], F32R, tag="tmp")
                            nc.vector.tensor_mul(tmp[:], ps_c[0:64, :], rbb[:])
                            nc.gpsimd.dma_start(out=ctxn[64:128, :], in_=tmp[:])
                    # stage D: output projection (partial)
                    for ss in range(SC // 128):
                        o_sb = ost.tile([128, 1024], F32, tag="o")
                        for dc in range(2):
                            ps_o = ps_out.tile([128, 512], F32, tag="out")
                            nc.tensor.matmul(
                                ps_o[:],
                                ctxn[:, ss * 128:(ss + 1) * 128],
                                wo_sb[:, dc * 512:(dc + 1) * 512],
                                start=True, stop=True,
                            )
                            nc.vector.tensor_copy(o_sb[:, dc * 512:(dc + 1) * 512], ps_o[:])
                        nc.gpsimd.dma_start(
                            out=out[q0 + ss * 128:q0 + (ss + 1) * 128, :], in_=o_sb[:]
                        )
    nc.finalize()
    return nc


_NC_CACHE = None


def make_in_maps(x, Wq, Wk, Wv, bq, bk, bv, Wo, bo=None):
    xT = np.ascontiguousarray(x.reshape(TOK, D).T)  # [D, TOK]
    in_maps = []
    for c in range(NCORES):
        h0 = 2 * c
        in_maps.append({
            "xT": xT,
            "wq": np.ascontiguousarray(np.concatenate([Wq[h0], Wq[h0 + 1]], axis=1)),
            "wk": np.ascontiguousarray(np.concatenate([Wk[h0], Wk[h0 + 1]], axis=1)),
            "wv": np.ascontiguousarray(np.concatenate([Wv[h0], Wv[h0 + 1]], axis=1)),
            "bqkv": np.ascontiguousarray(np.stack([
                bq[h0:h0 + 2].reshape(DH2),
                bk[h0:h0 + 2].reshape(DH2),
                bv[h0:h0 + 2].reshape(DH2),
            ])),
            "wo": np.ascontiguousarray(Wo[c * DH2:(c + 1) * DH2]),
            "ones": np.ones((128, 32), dtype=np.float32),
            "iden": np.eye(128, dtype=np.float32),
        })
    return in_maps


def kernel(x, Wq, Wk, Wv, bq, bk, bv, Wo, bo):
    global _NC_CACHE
    if _NC_CACHE is None:
        _NC_CACHE = build_bass()
    nc = _NC_CACHE

    in_maps = make_in_maps(x, Wq, Wk, Wv, bq, bk, bv, Wo)
    res = run_bass_kernel_spmd(nc, in_maps, list(range(NCORES)))
    acc = np.zeros((TOK, D), dtype=np.float64)
    for c in range(NCORES):
        acc += res.results[c]["out"]
    acc += bo
    return acc.astype(np.float32).reshape(B, S, D)



# revision 10
# speedup vs baseline: 1.4004x; 1.4004x over previous
"""Multi-head attention (B=2, S=2048, D=1024, H=16, dh=64) on 8 TRN2 NeuronCores.

Sharding: tensor-parallel over heads - 2 heads per core. Each core computes
Q/K/V projections for its 2 heads, full attention over S=2048, and a partial
output projection (its 128 rows of Wo). Host sums the 8 partial outputs + bo.

All matmul operands are bf16 (1 cyc/row on PE); PSUM accumulation stays fp32.
Numerically validated on CPU: end-to-end bf16 gives rel err ~2.8e-3 vs the
2e-2 gate.

Per-core schedule (software-pipelined 3 deep, instruction-interleaved):
  A) QKV^T:   psum[dh2=128, s 512] = sum_kt W[128,128].T @ x[128,512]
              bias-add on ACT engine -> qT/kT/vT bf16
  T) V^T -> V[t, e] via PE transpose (interleaved into A(b1) + main iter 0)
  main iter i: per t-tile: B(i) scoresT mms | exp(i) on ACT | C(i-1) ctx mms
               | D(i-2) out-proj mms in back half
  N(i-1) after loop: 1/l via fast reciprocal, one DRAM-roundtrip partition
               broadcast, ctx normalize -> ctxn bf16 (consumed by D one
               iteration later, hiding the roundtrip latency)
"""

import os

import numpy as np
import ml_dtypes

import concourse.bacc as bacc
import concourse.mybir as mybir
import concourse.tile as tile
from concourse.bass_utils import run_bass_kernel_spmd

F32 = mybir.dt.float32
BF16 = mybir.dt.bfloat16
AF = mybir.ActivationFunctionType

B, S, D, H, DH = 2, 2048, 1024, 16, 64
TOK = B * S          # 4096
DH2 = 2 * DH         # 128 (two heads per core)
NCORES = 8
SC = 512             # s-chunk (queries per main block)
NSC = S // SC        # 4 s-chunks per batch
NT = S // 128        # 16 t-tiles per batch
NKT = D // 128       # 8 k-tiles of contraction
NCH = TOK // SC      # 8 token chunks for stage A
NBLK = B * NSC       # 8 main blocks

BF16NP = ml_dtypes.bfloat16


def build_bass():
    nc = bacc.Bacc(None, target_bir_lowering=False)

    xT = nc.dram_tensor("xT", [D, TOK], BF16, kind="ExternalInput")
    wq = nc.dram_tensor("wq", [D, DH2], BF16, kind="ExternalInput")
    wk = nc.dram_tensor("wk", [D, DH2], BF16, kind="ExternalInput")
    wv = nc.dram_tensor("wv", [D, DH2], BF16, kind="ExternalInput")
    bqkv = nc.dram_tensor("bqkv", [3, DH2], F32, kind="ExternalInput")
    wo = nc.dram_tensor("wo", [DH2, D], BF16, kind="ExternalInput")
    iden = nc.dram_tensor("iden", [128, 128], BF16, kind="ExternalInput")
    out = nc.dram_tensor("out", [TOK, D], BF16, kind="ExternalOutput")
    debug = os.environ.get("KDBG") == "1"
    if debug:
        dbg_r = nc.dram_tensor("dbg_r", [NBLK, 1024], F32, kind="ExternalOutput")
        dbg_b = nc.dram_tensor("dbg_b", [NBLK, 4, 512], F32, kind="ExternalOutput")
        dbg_c = nc.dram_tensor("dbg_c", [NBLK, 2, 512], BF16, kind="ExternalOutput")

    with tile.TileContext(nc) as tc:
        with (
            tc.tile_pool(name="persist", bufs=1) as persist,
            tc.tile_pool(name="xin", bufs=3) as xin,
            tc.tile_pool(name="exps", bufs=24) as exps,
            tc.tile_pool(name="work", bufs=2) as work,
            tc.tile_pool(name="ost", bufs=3) as ost,
            tc.tile_pool(name="ps", bufs=1, space="PSUM") as ps,
            tc.tile_pool(name="dscratch", bufs=2, space="DRAM") as dscratch,
        ):
            # ---- constants / persistent tiles ----
            w_sb = persist.tile([128, 3, NKT, DH2], BF16, tag="w")
            for i, w in enumerate((wq, wk, wv)):
                nc.sync.dma_start(
                    out=w_sb[:, i, :, :],
                    in_=w.rearrange("(t p) m -> p t m", p=128),
                )
            b_sb = persist.tile([128, 3], F32, tag="b")
            nc.gpsimd.dma_start(out=b_sb[:], in_=bqkv.rearrange("q p -> p q"))
            wo_sb = persist.tile([128, D], BF16, tag="wo")
            nc.sync.dma_start(out=wo_sb[:], in_=wo[:, :])
            ident = persist.tile([128, 128], BF16, tag="id")
            nc.sync.dma_start(out=ident[:], in_=iden[:, :])

            qT = persist.tile([128, TOK], BF16, tag="qT")
            kT = persist.tile([128, TOK], BF16, tag="kT")
            vT = persist.tile([128, TOK], BF16, tag="vT")
            # V in [t, e] layout, 130 = [V_h0(64) | 1 | V_h1(64) | 1]
            v_sb = persist.tile([128, B * NT, 130], BF16, tag="v")
            nc.gpsimd.memset(v_sb[:, :, 64], 1.0)
            nc.gpsimd.memset(v_sb[:, :, 129], 1.0)

            xTv = xT.rearrange("(t p) n -> p t n", p=128)

            def emit_transpose(blk):
                ps_t = ps.tile([128, 128], BF16, tag="out", bufs=2)
                nc.tensor.transpose(
                    ps_t[:], vT[:, blk * 128:(blk + 1) * 128], ident[:]
                )
                nc.vector.tensor_copy(v_sb[:, blk, 0:64], ps_t[:, 0:64])
                nc.vector.tensor_copy(v_sb[:, blk, 65:129], ps_t[:, 64:128])

            # ---- stage A: QKV projections (+ T(b0) interleaved) ----
            for ch in range(NCH):
                c0 = ch * SC
                x_t = xin.tile([128, NKT, SC], BF16, tag="x")
                eng = nc.scalar if ch % 2 == 0 else nc.sync
                eng.dma_start(out=x_t[:], in_=xTv[:, :, c0:c0 + SC])
                ps_qk = ps.tile([128, 1024], F32, tag="bs", bufs=1)
                ps_v = ps.tile([128, 512], F32, tag="out", bufs=2)
                outs = (ps_qk[:, 0:512], ps_qk[:, 512:1024], ps_v[:])
                dests = (qT, kT, vT)
                for p in range(3):
                    for kt in range(NKT):
                        nc.tensor.matmul(
                            outs[p],
                            w_sb[:, p, kt, :],
                            x_t[:, kt, :],
                            start=(kt == 0), stop=(kt == NKT - 1),
                        )
                for p in range(3):
                    nc.scalar.activation(
                        dests[p][:, c0:c0 + SC], outs[p],
                        AF.Identity, bias=b_sb[:, p:p + 1],
                    )
                if ch >= 4:  # T(b0) hidden under A(b1) compute
                    for k in range(4):
                        emit_transpose((ch - 4) * 4 + k)

            # ---- main loop: software pipeline B/E(i) | C(i-1) | D(i-2) ----
            def blk_bq(j):
                return j // NSC, (j // NSC) * S + (j % NSC) * SC

            etiles = {}
            ctxns = {}
            o_cur = {}

            for i in range(NBLK + 2):
                has_BE = i < NBLK
                has_C = 1 <= i <= NBLK
                has_D = 2 <= i
                cb, db = i - 1, i - 2
                if has_BE:
                    b_i, q0_i = blk_bq(i)
                if has_C:
                    b_c = cb // NSC
                    ps_c0 = ps.tile([65, SC], F32, tag="ctx", bufs=4)
                    ps_c1 = ps.tile([65, SC], F32, tag="ctx", bufs=4)
                if has_D:
                    b_d, q0_d = blk_bq(db)

                for tt in range(NT):
                    if has_BE:
                        t0 = b_i * S + tt * 128
                        ps_s = ps.tile([128, 1024], F32, tag="bs", bufs=1)
                        nc.tensor.matmul(
                            ps_s[:, 0:512],
                            kT[0:64, t0:t0 + 128],
                            qT[0:64, q0_i:q0_i + SC],
                            start=True, stop=True,
                        )
                        nc.tensor.matmul(
                            ps_s[:, 512:1024],
                            kT[64:128, t0:t0 + 128],
                            qT[64:128, q0_i:q0_i + SC],
                            start=True, stop=True,
                        )
                        e_t = exps.tile([128, 1024], BF16, tag="e")
                        etiles[(i, tt)] = e_t
                        nc.scalar.activation(e_t[:], ps_s[:], AF.Exp, scale=0.125)
                    if i == 0:  # T(b1) hidden in prologue gaps
                        emit_transpose(NT + tt)
                    if has_C:
                        e_c = etiles.pop((cb, tt))
                        vblk = b_c * NT + tt
                        nc.tensor.matmul(
                            ps_c0[:],
                            v_sb[:, vblk, 0:65],
                            e_c[:, 0:512],
                            start=(tt == 0), stop=(tt == NT - 1),
                            skip_group_check=True,
                        )
                        nc.tensor.matmul(
                            ps_c1[:],
                            v_sb[:, vblk, 65:130],
                            e_c[:, 512:1024],
                            start=(tt == 0), stop=(tt == NT - 1),
                            skip_group_check=True,
                        )
                    if has_D and tt >= 8:
                        j = tt - 8
                        ss, dc = j // 2, j % 2
                        if dc == 0:
                            o_cur[ss] = ost.tile([128, 1024], BF16, tag="o",
                                                 name="o_sb")
                        o_sb = o_cur[ss]
                        ps_o = ps.tile([128, 512], F32, tag="out", bufs=2)
                        nc.tensor.matmul(
                            ps_o[:],
                            ctxns[db][:, ss * 128:(ss + 1) * 128],
                            wo_sb[:, dc * 512:(dc + 1) * 512],
                            start=True, stop=True,
                        )
                        nc.vector.tensor_copy(o_sb[:, dc * 512:(dc + 1) * 512], ps_o[:])
                        if dc == 1:
                            nc.sync.dma_start(
                                out=out[q0_d + ss * 128:q0_d + (ss + 1) * 128, :],
                                in_=o_sb[:],
                            )

                if has_C:  # N(cb): denominators + normalize (latency-hidden)
                    # custom-DVE ops ignore AP base partitions, so shift l
                    # from psum partition 64 to partition 0 with native
                    # copies, then run the fast reciprocal fully aligned.
                    l2 = work.tile([1, 1024], F32, tag="l2")
                    nc.vector.tensor_copy(l2[:, 0:512], ps_c0[64:65, :])
                    nc.vector.tensor_copy(l2[:, 512:1024], ps_c1[64:65, :])
                    rb = work.tile([1, 1024], F32, tag="rb")
                    nc.vector.reciprocal_approx_fast(rb[:], l2[:])
                    rd = dscratch.tile([1, 1024], F32, tag="rd")
                    nc.gpsimd.dma_start(out=rd[:], in_=rb[:])
                    rbb0 = work.tile([64, SC], F32, tag="rbb0")
                    rbb1 = work.tile([64, SC], F32, tag="rbb1")
                    nc.gpsimd.dma_start(
                        out=rbb0[:], in_=rd[0:1, 0:512].to_broadcast([64, SC])
                    )
                    nc.gpsimd.dma_start(
                        out=rbb1[:], in_=rd[0:1, 512:1024].to_broadcast([64, SC])
                    )
                    ctxn = work.tile([128, SC], BF16, tag="ctxn", bufs=3)
                    nc.vector.tensor_mul(ctxn[0:64, :], ps_c0[0:64, :], rbb0[:])
                    tmp = work.tile([64, SC], BF16, tag="tmp")
                    nc.vector.tensor_mul(tmp[:], ps_c1[0:64, :], rbb1[:])
                    nc.gpsimd.dma_start(out=ctxn[64:128, :], in_=tmp[:])
                    ctxns[cb] = ctxn
                    if debug:
                        nc.sync.dma_start(out=dbg_r[cb:cb + 1, :], in_=rb[:])
                        nc.sync.dma_start(out=dbg_b[cb, 0:1, :], in_=rbb0[0:1, :])
                        nc.sync.dma_start(out=dbg_b[cb, 1:2, :], in_=rbb0[63:64, :])
                        nc.sync.dma_start(out=dbg_b[cb, 2:3, :], in_=rbb1[0:1, :])
                        nc.sync.dma_start(out=dbg_b[cb, 3:4, :], in_=rbb1[63:64, :])
                        nc.sync.dma_start(out=dbg_c[cb, 0:1, :], in_=ctxn[0:1, :])
                        nc.sync.dma_start(out=dbg_c[cb, 1:2, :], in_=ctxn[64:65, :])

    nc.finalize()
    return nc


_NC_CACHE = None


def make_in_maps(x, Wq, Wk, Wv, bq, bk, bv, Wo, bo=None):
    xT = np.ascontiguousarray(
        x.reshape(TOK, D).T.astype(BF16NP))  # [D, TOK] bf16
    iden = np.eye(128, dtype=BF16NP)
    in_maps = []
    for c in range(NCORES):
        h0 = 2 * c
        in_maps.append({
            "xT": xT,
            "wq": np.ascontiguousarray(
                np.concatenate([Wq[h0], Wq[h0 + 1]], axis=1).astype(BF16NP)),
            "wk": np.ascontiguousarray(
                np.concatenate([Wk[h0], Wk[h0 + 1]], axis=1).astype(BF16NP)),
            "wv": np.ascontiguousarray(
                np.concatenate([Wv[h0], Wv[h0 + 1]], axis=1).astype(BF16NP)),
            "bqkv": np.ascontiguousarray(np.stack([
                bq[h0:h0 + 2].reshape(DH2),
                bk[h0:h0 + 2].reshape(DH2),
                bv[h0:h0 + 2].reshape(DH2),
            ]).astype(np.float32)),
            "wo": np.ascontiguousarray(
                Wo[c * DH2:(c + 1) * DH2].astype(BF16NP)),
            "iden": iden,
        })
    return in_maps


def kernel(x, Wq, Wk, Wv, bq, bk, bv, Wo, bo):
    global _NC_CACHE
    if _NC_CACHE is None:
        _NC_CACHE = build_bass()
    nc = _NC_CACHE

    in_maps = make_in_maps(x, Wq, Wk, Wv, bq, bk, bv, Wo)
    res = run_bass_kernel_spmd(nc, in_maps, list(range(NCORES)))
    acc = np.zeros((TOK, D), dtype=np.float64)
    for c in range(NCORES):
        acc += res.results[c]["out"].astype(np.float64)
    acc += bo
    return acc.astype(np.float32).reshape(B, S, D)


# revision 11
# speedup vs baseline: 1.7693x; 1.2634x over previous
"""Multi-head attention (B=2, S=2048, D=1024, H=16, dh=64) on 8 TRN2 NeuronCores.

Sharding: tensor-parallel over heads - 2 heads per core. Each core computes
Q/K/V projections for its 2 heads, full attention over S=2048, and a partial
output projection (its 128 rows of Wo). Host sums the 8 partial outputs + bo.

All matmul operands are bf16 (1 cyc/row on PE); PSUM accumulation stays fp32.
Numerically validated on CPU: end-to-end bf16 gives rel err ~2.8e-3 vs the
2e-2 gate.

Per-core schedule (software-pipelined 3 deep, instruction-interleaved):
  A) QKV^T:   psum[dh2=128, s 512] = sum_kt W[128,128].T @ x[128,512]
              bias-add on ACT engine -> qT/kT/vT bf16
  T) V^T -> V[t, e] via PE transpose (interleaved into A(b1) + main iter 0)
  main iter i: per t-tile: B(i) scoresT mms | exp(i) on ACT | C(i-1) ctx mms
               | D(i-2) out-proj mms in back half
  N(i-1) after loop: 1/l via fast reciprocal, one DRAM-roundtrip partition
               broadcast, ctx normalize -> ctxn bf16 (consumed by D one
               iteration later, hiding the roundtrip latency)
"""

import os

import numpy as np
import ml_dtypes

import concourse.bacc as bacc
import concourse.mybir as mybir
import concourse.tile as tile
from concourse.bass_utils import run_bass_kernel_spmd

F32 = mybir.dt.float32
BF16 = mybir.dt.bfloat16
AF = mybir.ActivationFunctionType

B, S, D, H, DH = 2, 2048, 1024, 16, 64
TOK = B * S          # 4096
DH2 = 2 * DH         # 128 (two heads per core)
NCORES = 8
SC = 512             # s-chunk (queries per main block)
NSC = S // SC        # 4 s-chunks per batch
NT = S // 128        # 16 t-tiles per batch
NKT = D // 128       # 8 k-tiles of contraction
NCH = TOK // SC      # 8 token chunks for stage A
NBLK = B * NSC       # 8 main blocks

BF16NP = ml_dtypes.bfloat16


def build_bass():
    nc = bacc.Bacc(None, target_bir_lowering=False)

    xT = nc.dram_tensor("xT", [D, TOK], BF16, kind="ExternalInput")
    wq = nc.dram_tensor("wq", [D, DH2], BF16, kind="ExternalInput")
    wk = nc.dram_tensor("wk", [D, DH2], BF16, kind="ExternalInput")
    wv = nc.dram_tensor("wv", [D, DH2], BF16, kind="ExternalInput")
    bqkv = nc.dram_tensor("bqkv", [3, DH2], F32, kind="ExternalInput")
    wo = nc.dram_tensor("wo", [DH2, D], BF16, kind="ExternalInput")
    iden = nc.dram_tensor("iden", [128, 128], BF16, kind="ExternalInput")
    out = nc.dram_tensor("out", [TOK, D], BF16, kind="ExternalOutput")
    debug = os.environ.get("KDBG") == "1"
    if debug:
        dbg_r = nc.dram_tensor("dbg_r", [NBLK, 1024], F32, kind="ExternalOutput")
        dbg_b = nc.dram_tensor("dbg_b", [NBLK, 4, 512], F32, kind="ExternalOutput")
        dbg_c = nc.dram_tensor("dbg_c", [NBLK, 2, 512], BF16, kind="ExternalOutput")

    with tile.TileContext(nc) as tc:
        with (
            tc.tile_pool(name="persist", bufs=1) as persist,
            tc.tile_pool(name="xin", bufs=3) as xin,
            tc.tile_pool(name="exps", bufs=24) as exps,
            tc.tile_pool(name="work", bufs=2) as work,
            tc.tile_pool(name="ost", bufs=3) as ost,
            tc.tile_pool(name="ps", bufs=1, space="PSUM") as ps,
            tc.tile_pool(name="dscratch", bufs=2, space="DRAM") as dscratch,
        ):
            # ---- constants / persistent tiles ----
            w_sb = persist.tile([128, 3, NKT, DH2], BF16, tag="w")
            for i, w in enumerate((wq, wk, wv)):
                nc.gpsimd.dma_start(
                    out=w_sb[:, i, :, :],
                    in_=w.rearrange("(t p) m -> p t m", p=128),
                )
            b_sb = persist.tile([128, 3], F32, tag="b")
            nc.gpsimd.dma_start(out=b_sb[:], in_=bqkv.rearrange("q p -> p q"))
            wo_sb = persist.tile([128, D], BF16, tag="wo")
            nc.gpsimd.dma_start(out=wo_sb[:], in_=wo[:, :])
            ident = persist.tile([128, 128], BF16, tag="id")
            nc.gpsimd.dma_start(out=ident[:], in_=iden[:, :])

            qT = persist.tile([128, TOK], BF16, tag="qT")
            kT = persist.tile([128, TOK], BF16, tag="kT")
            vT = persist.tile([128, TOK], BF16, tag="vT")
            # V in [t, e] layout, 130 = [V_h0(64) | 1 | V_h1(64) | 1]
            v_sb = persist.tile([128, B * NT, 130], BF16, tag="v")
            nc.gpsimd.memset(v_sb[:, :, 64], 1.0)
            nc.gpsimd.memset(v_sb[:, :, 129], 1.0)

            xTv = xT.rearrange("(t p) n -> p t n", p=128)

            def emit_transpose(blk):
                ps_t = ps.tile([128, 128], BF16, tag="out", bufs=1)
                nc.tensor.transpose(
                    ps_t[:], vT[:, blk * 128:(blk + 1) * 128], ident[:]
                )
                nc.vector.tensor_copy(v_sb[:, blk, 0:64], ps_t[:, 0:64])
                nc.vector.tensor_copy(v_sb[:, blk, 65:129], ps_t[:, 64:128])

            # ---- stage A: QKV projections (+ T(b0) interleaved) ----
            for ch in range(NCH):
                c0 = ch * SC
                x_t = xin.tile([128, NKT, SC], BF16, tag="x")
                eng = nc.scalar if ch % 2 == 0 else nc.sync
                eng.dma_start(out=x_t[:], in_=xTv[:, :, c0:c0 + SC])
                ps_qk = ps.tile([128, 1024], F32, tag="bs", bufs=2)
                ps_v = ps.tile([128, 512], F32, tag="out", bufs=1)
                outs = (ps_qk[:, 0:512], ps_qk[:, 512:1024], ps_v[:])
                dests = (qT, kT, vT)
                for p in range(3):
                    for kt in range(NKT):
                        nc.tensor.matmul(
                            outs[p],
                            w_sb[:, p, kt, :],
                            x_t[:, kt, :],
                            start=(kt == 0), stop=(kt == NKT - 1),
                        )
                for p in range(3):
                    nc.scalar.activation(
                        dests[p][:, c0:c0 + SC], outs[p],
                        AF.Identity, bias=b_sb[:, p:p + 1],
                    )
                if ch >= 4:  # T(b0) hidden under A(b1) compute
                    for k in range(4):
                        emit_transpose((ch - 4) * 4 + k)

            # ---- main loop: software pipeline B/E(i) | C(i-1) | D(i-2) ----
            def blk_bq(j):
                return j // NSC, (j // NSC) * S + (j % NSC) * SC

            etiles = {}
            ctxns = {}
            o_cur = {}

            for i in range(NBLK + 2):
                has_BE = i < NBLK
                has_C = 1 <= i <= NBLK
                has_D = 2 <= i
                cb, db = i - 1, i - 2
                if has_BE:
                    b_i, q0_i = blk_bq(i)
                if has_C:
                    b_c = cb // NSC
                    ps_c0 = ps.tile([65, SC], F32, tag="ctx", bufs=3)
                    ps_c1 = ps.tile([65, SC], F32, tag="ctx", bufs=3)
                if has_D:
                    b_d, q0_d = blk_bq(db)

                for tt in range(NT):
                    if has_BE:
                        t0 = b_i * S + tt * 128
                        ps_s = ps.tile([128, 1024], F32, tag="bs", bufs=2)
                        nc.tensor.matmul(
                            ps_s[:, 0:512],
                            kT[0:64, t0:t0 + 128],
                            qT[0:64, q0_i:q0_i + SC],
                            start=True, stop=True,
                        )
                        nc.tensor.matmul(
                            ps_s[:, 512:1024],
                            kT[64:128, t0:t0 + 128],
                            qT[64:128, q0_i:q0_i + SC],
                            start=True, stop=True,
                        )
                        e_t = exps.tile([128, 1024], BF16, tag="e")
                        etiles[(i, tt)] = e_t
                        nc.scalar.activation(e_t[:], ps_s[:], AF.Exp, scale=0.125)
                    if i == 0:  # T(b1) hidden in prologue gaps
                        emit_transpose(NT + tt)
                    if has_C:
                        e_c = etiles.pop((cb, tt))
                        vblk = b_c * NT + tt
                        nc.tensor.matmul(
                            ps_c0[:],
                            v_sb[:, vblk, 0:65],
                            e_c[:, 0:512],
                            start=(tt == 0), stop=(tt == NT - 1),
                            skip_group_check=True,
                        )
                        nc.tensor.matmul(
                            ps_c1[:],
                            v_sb[:, vblk, 65:130],
                            e_c[:, 512:1024],
                            start=(tt == 0), stop=(tt == NT - 1),
                            skip_group_check=True,
                        )
                    if has_D and tt >= 8:
                        j = tt - 8
                        ss, dc = j // 2, j % 2
                        if dc == 0:
                            o_cur[ss] = ost.tile([128, 1024], BF16, tag="o",
                                                 name="o_sb")
                        o_sb = o_cur[ss]
                        ps_o = ps.tile([128, 512], F32, tag="out", bufs=1)
                        nc.tensor.matmul(
                            ps_o[:],
                            ctxns[db][:, ss * 128:(ss + 1) * 128],
                            wo_sb[:, dc * 512:(dc + 1) * 512],
                            start=True, stop=True,
                        )
                        nc.vector.tensor_copy(o_sb[:, dc * 512:(dc + 1) * 512], ps_o[:])
                        if dc == 1:
                            nc.sync.dma_start(
                                out=out[q0_d + ss * 128:q0_d + (ss + 1) * 128, :],
                                in_=o_sb[:],
                            )

                if has_C:  # N(cb): denominators + normalize (latency-hidden)
                    # custom-DVE ops ignore AP base partitions, so shift l
                    # from psum partition 64 to partition 0 with native
                    # copies, then run the fast reciprocal fully aligned.
                    l2 = work.tile([1, 1024], F32, tag="l2")
                    nc.vector.tensor_copy(l2[:, 0:512], ps_c0[64:65, :])
                    nc.vector.tensor_copy(l2[:, 512:1024], ps_c1[64:65, :])
                    rb = work.tile([1, 1024], F32, tag="rb")
                    nc.vector.reciprocal_approx_fast(rb[:], l2[:])
                    rd = dscratch.tile([1, 1024], F32, tag="rd")
                    nc.gpsimd.dma_start(out=rd[:], in_=rb[:])
                    rbb0 = work.tile([64, SC], F32, tag="rbb0")
                    rbb1 = work.tile([64, SC], F32, tag="rbb1")
                    nc.gpsimd.dma_start(
                        out=rbb0[:], in_=rd[0:1, 0:512].to_broadcast([64, SC])
                    )
                    nc.gpsimd.dma_start(
                        out=rbb1[:], in_=rd[0:1, 512:1024].to_broadcast([64, SC])
                    )
                    ctxn = work.tile([128, SC], BF16, tag="ctxn", bufs=3)
                    nc.vector.tensor_mul(ctxn[0:64, :], ps_c0[0:64, :], rbb0[:])
                    tmp = work.tile([64, SC], BF16, tag="tmp")
                    nc.vector.tensor_mul(tmp[:], ps_c1[0:64, :], rbb1[:])
                    nc.gpsimd.dma_start(out=ctxn[64:128, :], in_=tmp[:])
                    ctxns[cb] = ctxn
                    if debug:
                        nc.sync.dma_start(out=dbg_r[cb:cb + 1, :], in_=rb[:])
                        nc.sync.dma_start(out=dbg_b[cb, 0:1, :], in_=rbb0[0:1, :])
                        nc.sync.dma_start(out=dbg_b[cb, 1:2, :], in_=rbb0[63:64, :])
                        nc.sync.dma_start(out=dbg_b[cb, 2:3, :], in_=rbb1[0:1, :])
                        nc.sync.dma_start(out=dbg_b[cb, 3:4, :], in_=rbb1[63:64, :])
                        nc.sync.dma_start(out=dbg_c[cb, 0:1, :], in_=ctxn[0:1, :])
                        nc.sync.dma_start(out=dbg_c[cb, 1:2, :], in_=ctxn[64:65, :])

    nc.finalize()
    return nc


_NC_CACHE = None


def make_in_maps(x, Wq, Wk, Wv, bq, bk, bv, Wo, bo=None):
    xT = np.ascontiguousarray(
        x.reshape(TOK, D).T.astype(BF16NP))  # [D, TOK] bf16
    iden = np.eye(128, dtype=BF16NP)
    in_maps = []
    for c in range(NCORES):
        h0 = 2 * c
        in_maps.append({
            "xT": xT,
            "wq": np.ascontiguousarray(
                np.concatenate([Wq[h0], Wq[h0 + 1]], axis=1).astype(BF16NP)),
            "wk": np.ascontiguousarray(
                np.concatenate([Wk[h0], Wk[h0 + 1]], axis=1).astype(BF16NP)),
            "wv": np.ascontiguousarray(
                np.concatenate([Wv[h0], Wv[h0 + 1]], axis=1).astype(BF16NP)),
            "bqkv": np.ascontiguousarray(np.stack([
                bq[h0:h0 + 2].reshape(DH2),
                bk[h0:h0 + 2].reshape(DH2),
                bv[h0:h0 + 2].reshape(DH2),
            ]).astype(np.float32)),
            "wo": np.ascontiguousarray(
                Wo[c * DH2:(c + 1) * DH2].astype(BF16NP)),
            "iden": iden,
        })
    return in_maps


def kernel(x, Wq, Wk, Wv, bq, bk, bv, Wo, bo):
    global _NC_CACHE
    if _NC_CACHE is None:
        _NC_CACHE = build_bass()
    nc = _NC_CACHE

    in_maps = make_in_maps(x, Wq, Wk, Wv, bq, bk, bv, Wo)
    res = run_bass_kernel_spmd(nc, in_maps, list(range(NCORES)))
    acc = np.zeros((TOK, D), dtype=np.float64)
    for c in range(NCORES):
        acc += res.results[c]["out"].astype(np.float64)
    acc += bo
    return acc.astype(np.float32).reshape(B, S, D)


# revision 21
# speedup vs baseline: 1.8949x; 1.0710x over previous
"""Multi-head attention (B=2, S=2048, D=1024, H=16, dh=64) on 8 TRN2 NeuronCores.

Sharding: tensor-parallel over heads - 2 heads per core. Each core computes
Q/K/V projections for its 2 heads, full attention over S=2048, and a partial
output projection (its 128 rows of Wo). Host sums the 8 partial outputs + bo.

All matmul operands are bf16 (1 cyc/row on PE); PSUM accumulation stays fp32.
Numerically validated on CPU: end-to-end bf16 gives rel err ~2.8e-3 vs the
2e-2 gate.

Per-core schedule (software-pipelined 3 deep, instruction-interleaved):
  A) QKV^T:   psum[dh2=128, s 512] = sum_kt W[128,128].T @ x[128,512]
              bias-add on ACT engine -> qT/kT/vT bf16
  T) V^T -> V[t, e] via PE transpose (interleaved into A(b1) + main iter 0)
  main iter i: per t-tile: B(i) scoresT mms | exp(i) on ACT | C(i-1) ctx mms
               | D(i-2) out-proj mms in back half
  N(i-1) after loop: 1/l via fast reciprocal, one DRAM-roundtrip partition
               broadcast, ctx normalize -> ctxn bf16 (consumed by D one
               iteration later, hiding the roundtrip latency)
"""

import os

import numpy as np
import ml_dtypes

import concourse.bacc as bacc
import concourse.mybir as mybir
import concourse.tile as tile
from concourse.bass_utils import run_bass_kernel_spmd

F32 = mybir.dt.float32
BF16 = mybir.dt.bfloat16
AF = mybir.ActivationFunctionType

B, S, D, H, DH = 2, 2048, 1024, 16, 64
TOK = B * S          # 4096
DH2 = 2 * DH         # 128 (two heads per core)
NCORES = 8
SC = 512             # s-chunk (queries per main block)
NSC = S // SC        # 4 s-chunks per batch
NT = S // 128        # 16 t-tiles per batch
NKT = D // 128       # 8 k-tiles of contraction
NCH = TOK // SC      # 8 token chunks for stage A
NBLK = B * NSC       # 8 main blocks

BF16NP = ml_dtypes.bfloat16


def build_bass():
    nc = bacc.Bacc(None, target_bir_lowering=False)

    # xh: host pre-shuffled so each chunk is one contiguous 8KB/partition DMA
    xh = nc.dram_tensor("xh", [128, NCH, NKT, SC], BF16, kind="ExternalInput")
    # wqkv: host pre-shuffled [p, proj, kt, dh2], contiguous per partition
    wqkv = nc.dram_tensor("wqkv", [128, 3, NKT, DH2], BF16,
                          kind="ExternalInput")
    bqkv = nc.dram_tensor("bqkv", [128, 3], F32, kind="ExternalInput")
    wo = nc.dram_tensor("wo", [DH2, D], BF16, kind="ExternalInput")
    iden = nc.dram_tensor("iden", [128, 128], BF16, kind="ExternalInput")
    out = nc.dram_tensor("out", [TOK, D], BF16, kind="ExternalOutput")
    debug = os.environ.get("KDBG") == "1"
    if debug:
        dbg_r = nc.dram_tensor("dbg_r", [NBLK, 1024], F32, kind="ExternalOutput")
        dbg_b = nc.dram_tensor("dbg_b", [NBLK, 4, 512], F32, kind="ExternalOutput")
        dbg_c = nc.dram_tensor("dbg_c", [NBLK, 2, 512], BF16, kind="ExternalOutput")

    with tile.TileContext(nc) as tc:
        with (
            tc.tile_pool(name="persist", bufs=1) as persist,
            tc.tile_pool(name="xin", bufs=4) as xin,
            tc.tile_pool(name="exps", bufs=24) as exps,
            tc.tile_pool(name="work", bufs=2) as work,
            tc.tile_pool(name="ost", bufs=3) as ost,
            tc.tile_pool(name="ps", bufs=1, space="PSUM") as ps,
            tc.tile_pool(name="dscratch", bufs=2, space="DRAM") as dscratch,
        ):
            # ---- constants / persistent tiles ----
            # weights + small constants on the sync queue; x halves use
            # scalar/gpsimd first so the first chunk lands fast
            w_sb = persist.tile([128, 3, NKT, DH2], BF16, tag="w")
            nc.sync.dma_start(out=w_sb[:], in_=wqkv[:])
            b_sb = persist.tile([128, 3], F32, tag="b")
            nc.sync.dma_start(out=b_sb[:], in_=bqkv[:])
            ident = persist.tile([128, 128], BF16, tag="id")
            nc.sync.dma_start(out=ident[:], in_=iden[:, :])
            wo_sb = persist.tile([128, D], BF16, tag="wo")
            nc.sync.dma_start(out=wo_sb[:], in_=wo[:, :])

            qT = persist.tile([128, TOK], BF16, tag="qT")
            kT = persist.tile([128, TOK], BF16, tag="kT")
            vT = persist.tile([128, TOK], BF16, tag="vT")
            # V in [t, e] layout, 130 = [V_h0(64) | 1 | V_h1(64) | 1]
            v_sb = persist.tile([128, B * NT, 130], BF16, tag="v")
            nc.gpsimd.memset(v_sb[:, :, 64], 1.0)
            nc.gpsimd.memset(v_sb[:, :, 129], 1.0)

            def emit_transpose(blk):
                ps_t = ps.tile([128, 128], BF16, tag="out", bufs=1)
                nc.tensor.transpose(
                    ps_t[:], vT[:, blk * 128:(blk + 1) * 128], ident[:]
                )
                nc.vector.tensor_copy(v_sb[:, blk, 0:64], ps_t[:, 0:64])
                nc.vector.tensor_copy(v_sb[:, blk, 65:129], ps_t[:, 64:128])

            # ---- stage A: QKV projections (+ T(b0) interleaved) ----
            # each chunk split into two half-DMAs on different queues so
            # multiple DMA rings run in parallel (per-ring bw ~90 GB/s)
            xq = [(nc.scalar, nc.gpsimd), (nc.gpsimd, nc.scalar),
                  (nc.scalar, nc.gpsimd), (nc.sync, nc.scalar),
                  (nc.gpsimd, nc.sync), (nc.scalar, nc.gpsimd),
                  (nc.sync, nc.scalar), (nc.gpsimd, nc.sync)]
            for ch in range(NCH):
                c0 = ch * SC
                x_t = xin.tile([128, NKT, SC], BF16, tag="x")
                e0, e1 = xq[ch]
                e0.dma_start(out=x_t[:, 0:4, :], in_=xh[:, ch, 0:4, :])
                e1.dma_start(out=x_t[:, 4:8, :], in_=xh[:, ch, 4:8, :])
                ps_qk = ps.tile([128, 1024], F32, tag="bs", bufs=2)
                ps_v = ps.tile([128, 512], F32, tag="out", bufs=1)
                outs = (ps_qk[:, 0:512], ps_qk[:, 512:1024], ps_v[:])
                dests = (qT, kT, vT)
                for p in range(3):
                    for kt in range(NKT):
                        nc.tensor.matmul(
                            outs[p],
                            w_sb[:, p, kt, :],
                            x_t[:, kt, :],
                            start=(kt == 0), stop=(kt == NKT - 1),
                        )
                for p in range(3):
                    nc.scalar.activation(
                        dests[p][:, c0:c0 + SC], outs[p],
                        AF.Identity, bias=b_sb[:, p:p + 1],
                    )
                if ch >= 4:  # T(b0) hidden under A(b1) compute
                    for k in range(4):
                        emit_transpose((ch - 4) * 4 + k)

            # ---- main loop: software pipeline B/E(i) | C(i-1) | D(i-2) ----
            def blk_bq(j):
                return j // NSC, (j // NSC) * S + (j % NSC) * SC

            etiles = {}
            ctxns = {}
            o_cur = {}

            for i in range(NBLK + 2):
                has_BE = i < NBLK
                has_C = 1 <= i <= NBLK
                has_D = 2 <= i
                cb, db = i - 1, i - 2
                if has_BE:
                    b_i, q0_i = blk_bq(i)
                if has_C:
                    b_c = cb // NSC
                    ps_c0 = ps.tile([65, SC], F32, tag="ctx", bufs=3)
                    ps_c1 = ps.tile([65, SC], F32, tag="ctx", bufs=3)
                if has_D:
                    b_d, q0_d = blk_bq(db)

                for tt in range(NT):
                    if has_BE:
                        t0 = b_i * S + tt * 128
                        ps_s = ps.tile([128, 1024], F32, tag="bs", bufs=2)
                        nc.tensor.matmul(
                            ps_s[:, 0:512],
                            kT[0:64, t0:t0 + 128],
                            qT[0:64, q0_i:q0_i + SC],
                            start=True, stop=True,
                        )
                        nc.tensor.matmul(
                            ps_s[:, 512:1024],
                            kT[64:128, t0:t0 + 128],
                            qT[64:128, q0_i:q0_i + SC],
                            start=True, stop=True,
                        )
                        e_t = exps.tile([128, 1024], BF16, tag="e")
                        etiles[(i, tt)] = e_t
                        nc.scalar.activation(e_t[:], ps_s[:], AF.Exp, scale=0.125)
                    if i == 0:  # T(b1) hidden in prologue gaps
                        emit_transpose(NT + tt)
                    if has_C:
                        e_c = etiles.pop((cb, tt))
                        vblk = b_c * NT + tt
                        nc.tensor.matmul(
                            ps_c0[:],
                            v_sb[:, vblk, 0:65],
                            e_c[:, 0:512],
                            start=(tt == 0), stop=(tt == NT - 1),
                            skip_group_check=True,
                        )
                        nc.tensor.matmul(
                            ps_c1[:],
                            v_sb[:, vblk, 65:130],
                            e_c[:, 512:1024],
                            start=(tt == 0), stop=(tt == NT - 1),
                            skip_group_check=True,
                        )
                    if has_D and tt >= 8:
                        j = tt - 8
                        ss, dc = j // 2, j % 2
                        if dc == 0:
                            o_cur[ss] = ost.tile([128, 1024], BF16, tag="o",
                                                 name="o_sb")
                        o_sb = o_cur[ss]
                        if i >= NBLK:
                            # epilogue: B is done, reuse its psum banks so
                            # back-to-back D matmuls don't stall on copies
                            if dc == 0:
                                o_cur["ps"] = ps.tile([128, 1024], F32,
                                                      tag="bs", bufs=2,
                                                      name="d_ps")
                            ps_o = o_cur["ps"][:, dc * 512:(dc + 1) * 512]
                        else:
                            ps_o = ps.tile([128, 512], F32, tag="out",
                                           bufs=1, name="ps_o")[:]
                        nc.tensor.matmul(
                            ps_o,
                            ctxns[db][:, ss * 128:(ss + 1) * 128],
                            wo_sb[:, dc * 512:(dc + 1) * 512],
                            start=True, stop=True,
                        )
                        nc.vector.tensor_copy(o_sb[:, dc * 512:(dc + 1) * 512], ps_o)
                        if dc == 1:
                            nc.sync.dma_start(
                                out=out[q0_d + ss * 128:q0_d + (ss + 1) * 128, :],
                                in_=o_sb[:],
                            )

                if has_C:  # N(cb): denominators + normalize (latency-hidden)
                    # custom-DVE ops ignore AP base partitions, so shift l
                    # from psum partition 64 to partition 0 with native
                    # copies, then run the fast reciprocal fully aligned.
                    l2 = work.tile([1, 1024], F32, tag="l2")
                    nc.vector.tensor_copy(l2[:, 0:512], ps_c0[64:65, :])
                    nc.vector.tensor_copy(l2[:, 512:1024], ps_c1[64:65, :])
                    rb = work.tile([1, 1024], F32, tag="rb")
                    nc.vector.reciprocal_approx_fast(rb[:], l2[:])
                    rd = dscratch.tile([1, 1024], F32, tag="rd")
                    nc.gpsimd.dma_start(out=rd[:], in_=rb[:])
                    rbb0 = work.tile([64, SC], F32, tag="rbb0")
                    rbb1 = work.tile([64, SC], F32, tag="rbb1")
                    nc.gpsimd.dma_start(
                        out=rbb0[:], in_=rd[0:1, 0:512].to_broadcast([64, SC])
                    )
                    nc.gpsimd.dma_start(
                        out=rbb1[:], in_=rd[0:1, 512:1024].to_broadcast([64, SC])
                    )
                    ctxn = work.tile([128, SC], BF16, tag="ctxn", bufs=3)
                    nc.vector.tensor_mul(ctxn[0:64, :], ps_c0[0:64, :], rbb0[:])
                    tmp = work.tile([64, SC], BF16, tag="tmp")
                    nc.vector.tensor_mul(tmp[:], ps_c1[0:64, :], rbb1[:])
                    nc.gpsimd.dma_start(out=ctxn[64:128, :], in_=tmp[:])
                    ctxns[cb] = ctxn
                    if debug:
                        nc.sync.dma_start(out=dbg_r[cb:cb + 1, :], in_=rb[:])
                        nc.sync.dma_start(out=dbg_b[cb, 0:1, :], in_=rbb0[0:1, :])
                        nc.sync.dma_start(out=dbg_b[cb, 1:2, :], in_=rbb0[63:64, :])
                        nc.sync.dma_start(out=dbg_b[cb, 2:3, :], in_=rbb1[0:1, :])
                        nc.sync.dma_start(out=dbg_b[cb, 3:4, :], in_=rbb1[63:64, :])
                        nc.sync.dma_start(out=dbg_c[cb, 0:1, :], in_=ctxn[0:1, :])
                        nc.sync.dma_start(out=dbg_c[cb, 1:2, :], in_=ctxn[64:65, :])

    nc.finalize()
    return nc


_NC_CACHE = None


def make_in_maps(x, Wq, Wk, Wv, bq, bk, bv, Wo, bo=None):
    # xh[p, ch, t, s] = xT[t*128+p, ch*512+s], contiguous per partition
    xT = x.reshape(TOK, D).T.astype(BF16NP)  # [D, TOK]
    xhs = np.ascontiguousarray(
        xT.reshape(NKT, 128, NCH, SC).transpose(1, 2, 0, 3))

    def wshuf(w):  # [D, DH2] -> [p, kt, DH2]
        return w.astype(BF16NP).reshape(NKT, 128, DH2).transpose(1, 0, 2)

    iden = np.eye(128, dtype=BF16NP)
    in_maps = []
    for c in range(NCORES):
        h0 = 2 * c
        wq = np.concatenate([Wq[h0], Wq[h0 + 1]], axis=1)
        wk = np.concatenate([Wk[h0], Wk[h0 + 1]], axis=1)
        wv = np.concatenate([Wv[h0], Wv[h0 + 1]], axis=1)
        in_maps.append({
            "xh": xhs,
            "wqkv": np.ascontiguousarray(
                np.stack([wshuf(wq), wshuf(wk), wshuf(wv)], axis=1)),
            "bqkv": np.ascontiguousarray(np.stack([
                bq[h0:h0 + 2].reshape(DH2),
                bk[h0:h0 + 2].reshape(DH2),
                bv[h0:h0 + 2].reshape(DH2),
            ], axis=1).astype(np.float32)),
            "wo": np.ascontiguousarray(
                Wo[c * DH2:(c + 1) * DH2].astype(BF16NP)),
            "iden": iden,
        })
    return in_maps


def kernel(x, Wq, Wk, Wv, bq, bk, bv, Wo, bo):
    global _NC_CACHE
    if _NC_CACHE is None:
        _NC_CACHE = build_bass()
    nc = _NC_CACHE

    in_maps = make_in_maps(x, Wq, Wk, Wv, bq, bk, bv, Wo)
    res = run_bass_kernel_spmd(nc, in_maps, list(range(NCORES)))
    acc = np.zeros((TOK, D), dtype=np.float64)
    for c in range(NCORES):
        acc += res.results[c]["out"].astype(np.float64)
    acc += bo
    return acc.astype(np.float32).reshape(B, S, D)
